# revision 46
# baseline (speedup 1.0000x reference)
"""FP8ScaledLayer kernel for Trainium2 (8 NeuronCores, SPMD data-parallel).

Computes out = x @ (weight * scale[:, None]).T + bias with
  x: [4, 4096, 4096] fp32, weight: [4096, 4096] fp16,
  scale_weight: [4096] fp32, bias: [4096] fp32  ->  out [4, 4096, 4096] fp32.

Sharding: data-parallel over tokens (B*S = 16384 -> 2048 rows/core).
Weight is replicated; x is sharded, keeping every core compute-bound.

Design (evolved over several trace-driven iterations):
  - x path stays entirely on-chip: straight fp32 piece-loads into SBUF
    staging (split per-queue pools), DVE cast fp32->fp16, then PE transposes
    into the K-major resident xT/x8. The transposes are REGULAR matmuls
    against the identity (out = in.T @ I, fp32 PSUM, 4 groups of 8 per
    chunk) rather than transpose-mode: same result and cost, but they count
    as PE-busy for the HAM clock gate, so ramp bursts re-warm to 2.4GHz
    sooner (measured -7us). The XBAR must NOT be used for SBUF-source
    transposes: "DMA-transpose || SBUF->SBUF DMA" is a documented HW hazard
    and corrupts transfers under load. gpsimd SWDGE as a 3rd DMA queue was
    tested twice and produced intermittent NaN corruption both times --
    never use it here.
  - Mixed-precision K split: 24 of 32 k-chunks run fp16 matmuls, the last 8
    run as 4 fp8e4 DoubleRow matmuls (2x rate; each fp16->DR pair swap saves
    ~185ns/tile, ~24us total). Exact deterministic rel_err measured on HW
    with the real inputs: 1.890e-2 incl the bf16 output rounding (gate
    2e-2; offline e4m3 emulation predicts 1.866e-2 for the fp8 part alone
    and tracked the 6-chunk variant to ~1%; 9 chunks would land at
    ~1.98e-2 -- too close to the gate). The SBUF freed by the
    smaller xT is what lets the staging fit.
  - Weight tiles stream per 512-col pass: XBAR transposes (DRAM source only)
    on the sync queue, prep issued a full pass ahead; the fp8 ko's staged
    fp16 then DVE-quantized in one hop (longer sem chains stall pass starts).
  - scale/bias: HWDGE stride-0 broadcast to an fp32 transient, DVE-cast to
    bf16 replicas. (A gpsimd SWDGE cast+broadcast here intermittently
    corrupted one psum row of one tile -- rogue descriptor write.)
  - Queue roles: sync = weight XBAR transposes + half the x loads; scalar =
    other x loads + rep loads + PSUM->SBUF copies + output writes; DVE =
    casts + epilogue (psum*scale+bias); PE = matmuls + x transposes.
  - Interleaved (no 0,1) phase with no=0 running two tiles ahead (covers
    wt16(1) prep) and a staggered tail (STAG) so wT2's pool slot frees with
    ~37us of matmul cover; chunk production issued 4 ahead, self-paced by
    the staging pools.
"""

import sys

if "/opt/trn_rl_repo" not in sys.path:
    sys.path.insert(0, "/opt/trn_rl_repo")

import numpy as np

import concourse.bass as bass
import concourse.mybir as mybir
import concourse.tile as tile
from concourse import bacc
from concourse.masks import make_identity

P = 128
N_CORES = 8
B, S, K, N = 4, 4096, 4096, 4096
M_TOTAL = B * S
M_SH = M_TOTAL // N_CORES  # 2048 rows per core
KO = K // P  # 32
KO16 = 24  # k-chunks done in fp16
KO8 = KO - KO16  # 8 k-chunks done in fp8 DoubleRow (4 instructions)
KCUT = KO16 * P  # 3072
MO = M_SH // P  # 16
N_TILE = 512
NO = N // N_TILE  # 8
STAG = 6  # no=0 finishes STAG tiles early to give the wT2 transpose cover

F32 = mybir.dt.float32
F16 = mybir.dt.float16
BF16 = mybir.dt.bfloat16
F8 = mybir.dt.float8e4
DR = mybir.MatmulPerfMode.DoubleRow

_CACHED_NC = None


def _build_nc():
    nc = bacc.Bacc(
        None,
        target_bir_lowering=False,
        num_swdge_queues=1,
        dynamic_dma_scratch_size=2048,
    )

    x = nc.dram_tensor("x", (M_SH, K), F32, kind="ExternalInput")
    w = nc.dram_tensor("weight", (N, K), F16, kind="ExternalInput")
    scale = nc.dram_tensor("scale_weight", (N,), F32, kind="ExternalInput")
    bias = nc.dram_tensor("bias", (N,), F32, kind="ExternalInput")
    # Output is written bf16 and upcast to fp32 on the host (exact widening):
    # halves the output DMA bytes, relieving the HWDGE queues during the
    # x-supply-bound ramp. Costs ~2e-3 rel err in quadrature -> ~1.877e-2.
    out = nc.dram_tensor("out", (M_SH, N), BF16, kind="ExternalOutput")

    with tile.TileContext(nc) as tc:
        with (
            tc.tile_pool(name="xT", bufs=1) as xtp,
            tc.tile_pool(name="x8", bufs=1) as x8p,
            tc.tile_pool(name="wT", bufs=2) as wtp,
            tc.tile_pool(name="w8", bufs=2) as w8p,
            tc.tile_pool(name="x32s", bufs=4) as x32p,
            tc.tile_pool(name="x32y", bufs=3) as x32yp,
            tc.tile_pool(name="x16s", bufs=2) as x16p,
            tc.tile_pool(name="wf8s", bufs=1) as wf8p,
            tc.tile_pool(name="ident", bufs=1) as idp,
            tc.tile_pool(name="sbrep", bufs=2) as sbp,
            tc.tile_pool(name="reptmp", bufs=1) as rtp,
            tc.tile_pool(name="psum", bufs=4, space="PSUM") as pp,
            tc.tile_pool(name="psumT", bufs=2, space="PSUM") as ptp,
            tc.tile_pool(name="osb", bufs=2) as op,
        ):
            # resident transposed operands
            xT = xtp.tile((P, MO, KO16, P), F16)   # xT[p,mo,ko,m] = x16[mo*128+m, ko*128+p]
            x8 = x8p.tile((P, MO, KO8, P), F8)     # fp8 ko 24..31
            ident = idp.tile((P, P), F16)
            make_identity(nc, ident[:])
            wts = {}
            w8s = {}
            sreps = {}

            wf8s = {}

            def make_wf8(no):
                # fp8 part of the weight tile: XBAR-transpose ko 24..31 into an
                # fp16 staging tile (quantized later by quant_w8). All XBAR
                # transposes ride the sync queue: two queues driving the XBAR
                # concurrently corrupts transfers.
                wf8 = wf8p.tile((P, KO8, N_TILE), F16, tag="wf8")
                nc.sync.dma_start_transpose(
                    wf8[:], w[no * N_TILE:(no + 1) * N_TILE, KCUT:K]
                )
                wf8s[no] = wf8

            def make_wt16(no, pieces=2):
                # wT[p,ko,n] = w[no*512+n, ko*128+p]; fp16 part in `pieces`
                # XBAR transposes. (An 8-piece trickle for pass 0 starts the
                # first matmul at ~12us instead of ~23us but measured NET
                # SLOWER overall -- the extra sync kicks delay the chunk
                # banking this schedule relies on. Keep 2.)
                wTn = wtp.tile((P, KO16, N_TILE), F16, tag="wT")
                assert KO16 % pieces == 0
                sko = KO16 // pieces
                for j in range(pieces):
                    nc.sync.dma_start_transpose(
                        wTn[:, sko * j:sko * (j + 1), :],
                        w[no * N_TILE:(no + 1) * N_TILE,
                          sko * P * j:sko * P * (j + 1)],
                    )
                wts[no] = wTn

            def quant_w8(no):
                # DVE fp16 -> fp8e4; emitted separately so its queue position
                # (and the wf8 gate) never blocks epilogues for long.
                w8n = w8p.tile((P, KO8, N_TILE), F8, tag="w8")
                nc.vector.tensor_copy(w8n[:], wf8s[no][:])
                w8s[no] = w8n

            def make_wt(no):
                make_wf8(no)
                make_wt16(no)

            def make_reps(no):
                # HWDGE stride-0 partition broadcast into an fp32 transient,
                # DVE-cast to resident bf16 replicas (saves 4KB vs fp32 reps)
                s_rep = sbp.tile((P, N_TILE), BF16, tag="scale")
                b_rep = sbp.tile((P, N_TILE), BF16, tag="bias")
                for rep, src in ((s_rep, scale), (b_rep, bias)):
                    tmp = rtp.tile((P, N_TILE), F32, tag="rtmp")
                    sl = src[slice(no * N_TILE, (no + 1) * N_TILE)]
                    nc.scalar.dma_start(
                        out=tmp[:],
                        in_=bass.AP(tensor=sl.tensor, offset=sl.offset,
                                    ap=[[0, P], *sl.ap]),
                    )
                    nc.vector.tensor_copy(rep[:], tmp[:])
                sreps[no] = (s_rep, b_rep)

            def loads_chunk(mo, engs=(nc.scalar, nc.sync)):
                # x[mo*128:(mo+1)*128, :] fp32 loaded in 8 pieces (alternating
                # across the given HWDGE queues), DVE-cast to fp16 halves.
                # The ramp is x-supply-bound (every big PE gap in the trace
                # waits on these casts). Each queue's dma_start trigger waits
                # FIFO-blocking on a staging-pool slot (freed by the DVE cast
                # ~5us later), so per-queue throughput = slots-in-flight x
                # 262KB / recycle-latency. Per-queue SPLIT pools (3 scalar +
                # 2 sync slots) decouple the queues and deepen the scalar
                # pipeline. (gpsimd SWDGE as a 3rd queue was tested twice:
                # intermittent NaN corruption both times -- never use it.)
                rows = slice(mo * P, (mo + 1) * P)
                halves = []
                for h in range(2):
                    x16h = x16p.tile((P, K // 2), F16, tag="x16")
                    for q in range(4):
                        j = 4 * h + q
                        eng = engs[j % len(engs)]
                        pool = x32p if eng is nc.scalar else x32yp
                        x32 = pool.tile((P, 512), F32, tag="x32")
                        eng.dma_start(out=x32[:], in_=x[rows, 512 * j:512 * (j + 1)])
                        nc.vector.tensor_copy(
                            x16h[:, 512 * q:512 * (q + 1)], x32[:]
                        )
                    halves.append(x16h)
                return halves

            def produce_chunk(mo, all_scalar=False, halves=None):
                if halves is None:
                    halves = loads_chunk(
                        mo, engs=(nc.scalar,) if all_scalar else (nc.scalar, nc.sync)
                    )
                rows = slice(mo * P, (mo + 1) * P)
                # Transpose on the PE (is_transpose matmuls through PSUM):
                # XBAR cannot be used here -- a DMA-transpose with an SBUF
                # source is the documented "DMA-transpose || SBUF->SBUF DMA"
                # hazard and corrupts transfers under load.
                # half 0 = ko 0..15 (fp16); half 1 = ko 16..25 fp16 + 26..31 fp8
                # 4 groups of 8: regular matmuls against the identity
                # (out = in.T @ I, fp32 PSUM). Identical result/cost to
                # transpose-mode, but counts as PE-busy for the HAM clock
                # gate (transpose-mode doesn't), so ramp bursts re-warm to
                # 2.4GHz sooner. Groups: kos 0-7, 8-15, 16-23 -> xT (fp16),
                # kos 24-31 -> x8 (fp8).
                for g in range(4):
                    pt = ptp.tile((P, 8, P), F32, tag="pt")
                    half = halves[g // 2]
                    base = (g % 2) * 8
                    for j in range(8):
                        nc.tensor.matmul(
                            pt[:, j, :],
                            lhsT=half[:, P * (base + j):P * (base + j + 1)],
                            rhs=ident[:],
                            start=True,
                            stop=True,
                        )
                    if g < 3:
                        nc.scalar.copy(xT[:, mo, 8 * g:8 * (g + 1), :], pt[:])
                    else:
                        nc.vector.tensor_copy(x8[:, mo], pt[:])

            def mm_tile(mo, no):
                ncols = slice(no * N_TILE, (no + 1) * N_TILE)
                wT = wts[no]
                w8n = w8s[no]
                scale_rep, bias_rep = sreps[no]
                ps = pp.tile((P, N_TILE), F32, tag="ps")
                for ko in range(KO16):
                    nc.tensor.matmul(
                        ps[:],
                        lhsT=xT[:, mo, ko, :],
                        rhs=wT[:, ko, :],
                        start=(ko == 0),
                        stop=False,
                    )
                for j in range(KO8 // 2):
                    nc.tensor.matmul(
                        ps[:],
                        lhsT=x8[:, mo, 2 * j:2 * j + 2, :],
                        rhs=w8n[:, 2 * j:2 * j + 2, :],
                        start=False,
                        stop=(j == KO8 // 2 - 1),
                        perf_mode=DR,
                    )
                ot = op.tile((P, N_TILE), BF16, tag="ot")
                nc.vector.tensor_mul(ot[:], ps[:], scale_rep[:])
                nc.vector.tensor_add(ot[:], ot[:], bias_rep[:])
                nc.scalar.dma_start(out[mo * P:(mo + 1) * P, ncols], ot[:])

            def mm_tile_pair(mo, no, width=2):
                # `width` tiles with their fp16 runs back-to-back, then all
                # DR runs. The PE pays ~230-350ns at every normal<->DoubleRow
                # mode boundary (trace: first DR matmul spacing 403-566ns vs
                # the 215ns flow, ~1.3 events/tile = ~38us total); batching
                # divides the transitions by `width`. PSUM: width=2 keeps 2 banks
                # open + 2 draining. (width=4 measured +9us -- the full-bank
                # occupancy stalls the pipeline; keep 2.)
                ncols = slice(no * N_TILE, (no + 1) * N_TILE)
                wT = wts[no]
                w8n = w8s[no]
                scale_rep, bias_rep = sreps[no]
                pss = {}
                for m in range(mo, mo + width):
                    ps = pp.tile((P, N_TILE), F32, tag="ps", name="ps")
                    pss[m] = ps
                    for ko in range(KO16):
                        nc.tensor.matmul(
                            ps[:],
                            lhsT=xT[:, m, ko, :],
                            rhs=wT[:, ko, :],
                            start=(ko == 0),
                            stop=False,
                        )
                for m in range(mo, mo + width):
                    for j in range(KO8 // 2):
                        nc.tensor.matmul(
                            pss[m][:],
                            lhsT=x8[:, m, 2 * j:2 * j + 2, :],
                            rhs=w8n[:, 2 * j:2 * j + 2, :],
                            start=False,
                            stop=(j == KO8 // 2 - 1),
                            perf_mode=DR,
                        )
                for m in range(mo, mo + width):
                    ot = op.tile((P, N_TILE), BF16, tag="ot", name="ot")
                    nc.vector.tensor_mul(ot[:], pss[m][:], scale_rep[:])
                    nc.vector.tensor_add(ot[:], ot[:], bias_rep[:])
                    nc.scalar.dma_start(out[m * P:(m + 1) * P, ncols], ot[:])

            # ---- prologue: wf8(0) first (tiny, the DR matmuls ending tile
            # (0,0) need it), then wt16(0) / wt16(1) pieces on scalar while
            # sync builds chunks 0..3.
            # chunk 0/1 loads race ahead on both queues before the wT
            # transposes occupy sync; the wt16(0) wait (~30us) banks chunks.
            h0 = loads_chunk(0)
            make_wf8(0)
            make_wt16(0)
            quant_w8(0)  # after wt16(0): its wf8b transpose must not delay it
            produce_chunk(0, halves=h0)
            make_reps(0)
            produce_chunk(1, all_scalar=True)
            make_reps(1)
            produce_chunk(2, all_scalar=True)
            make_wt16(1)
            make_wf8(1)

            # ---- interleaved phase over no in {0,1}, with no=0 running two
            # tiles ahead so the first no=1 tile lands after wt16(1)'s
            # transposes. Chunk consumption is ~12.4us per pair; production is
            # issued 2-4 chunks ahead (~7.5us/chunk of queue time on each of
            # sync/scalar).
            mm_tile(0, 0)
            quant_w8(1)
            produce_chunk(3)
            mm_tile(1, 0)
            produce_chunk(4)
            for mo in range(MO - STAG):
                mm_tile(mo + 2, 0)
                mm_tile(mo, 1)
                if mo + 5 < MO:
                    produce_chunk(mo + 5)
                if mo >= 9:
                    produce_chunk(mo + 6)  # chunk 15 early
            # staggered tail: finish no=0 (tiles 12..15), so wT2's pool slot
            # frees with 6 no=1 tiles (~37us) of matmul cover.
            for mo in range(MO - STAG + 2, MO):
                mm_tile(mo, 0)
            make_wt(2)
            make_reps(2)
            for i, mo in enumerate(range(MO - STAG, MO)):
                mm_tile(mo, 1)
                if i == 2:
                    quant_w8(2)

            # ---- remaining passes; wT(no+1) + reps(no+1) issued at the start
            # of pass no so their transposes run with a full pass of cover;
            # the w8 quant a few tiles in so the wf8 gate never backs up the
            # DVE epilogue stream.
            for no in range(2, NO):
                if no + 1 < NO:
                    make_wt(no + 1)
                    make_reps(no + 1)
                for i, (mo, wd) in enumerate(((0, 3), (3, 3), (6, 3), (9, 3), (12, 2), (14, 2))):
                    mm_tile_pair(mo, no, width=wd)
                    if i == 1 and no + 1 < NO:
                        quant_w8(no + 1)

    nc.finalize()
    return nc


def _get_nc():
    global _CACHED_NC
    if _CACHED_NC is None:
        _CACHED_NC = _build_nc()
    return _CACHED_NC


def _run(inputs, trace=False, **spmd_kwargs):
    from concourse.bass_utils import run_bass_kernel_spmd

    x = np.asarray(inputs["x"], dtype=np.float32).reshape(M_TOTAL, K)
    w = np.ascontiguousarray(np.asarray(inputs["weight"], dtype=np.float16))
    scale = np.ascontiguousarray(np.asarray(inputs["scale_weight"], dtype=np.float32))
    bias = np.ascontiguousarray(np.asarray(inputs["bias"], dtype=np.float32))

    in_maps = []
    for c in range(N_CORES):
        in_maps.append(
            {
                "x": np.ascontiguousarray(x[c * M_SH:(c + 1) * M_SH]),
                "weight": w,
                "scale_weight": scale,
                "bias": bias,
            }
        )

    nc = _get_nc()
    res = run_bass_kernel_spmd(
        nc, in_maps, core_ids=list(range(N_CORES)), trace=trace, **spmd_kwargs
    )
    out = np.concatenate([res.results[c]["out"] for c in range(N_CORES)], axis=0).astype(np.float32)
    return out.reshape(B, S, N), res


def kernel(x, weight, scale_weight, bias):
    out, _ = _run({"x": x, "weight": weight, "scale_weight": scale_weight, "bias": bias})
    return out



# revision 47
# speedup vs baseline: 1.1717x; 1.1717x over previous
"""FP8ScaledLayer kernel for Trainium2 (8 NeuronCores, SPMD data-parallel).

Computes out = x @ (weight * scale[:, None]).T + bias with
  x: [4, 4096, 4096] fp32, weight: [4096, 4096] fp16,
  scale_weight: [4096] fp32, bias: [4096] fp32  ->  out [4, 4096, 4096] fp32.

Sharding: data-parallel over tokens (B*S = 16384 -> 2048 rows/core).
Weight is replicated; x is sharded, keeping every core compute-bound.

Design (evolved over several trace-driven iterations):
  - x path stays entirely on-chip: straight fp32 piece-loads into SBUF
    staging (split per-queue pools), DVE cast fp32->fp16, then PE transposes
    into the K-major resident xT/x8. The transposes are REGULAR matmuls
    against the identity (out = in.T @ I, fp32 PSUM, 4 groups of 8 per
    chunk) rather than transpose-mode: same result and cost, but they count
    as PE-busy for the HAM clock gate, so ramp bursts re-warm to 2.4GHz
    sooner (measured -7us). The XBAR must NOT be used for SBUF-source
    transposes: "DMA-transpose || SBUF->SBUF DMA" is a documented HW hazard
    and corrupts transfers under load. gpsimd SWDGE as a 3rd DMA queue was
    tested twice and produced intermittent NaN corruption both times --
    never use it here.
  - Mixed-precision K split: 24 of 32 k-chunks run fp16 matmuls, the last 8
    run as 4 fp8e4 DoubleRow matmuls (2x rate; each fp16->DR pair swap saves
    ~185ns/tile, ~24us total). Exact deterministic rel_err measured on HW
    with the real inputs: 1.890e-2 incl the bf16 output rounding (gate
    2e-2; offline e4m3 emulation predicts 1.866e-2 for the fp8 part alone
    and tracked the 6-chunk variant to ~1%; 9 chunks would land at
    ~1.98e-2 -- too close to the gate). The SBUF freed by the
    smaller xT is what lets the staging fit.
  - Weight tiles stream per 512-col pass: XBAR transposes (DRAM source only)
    on the sync queue, prep issued a full pass ahead; the fp8 ko's staged
    fp16 then DVE-quantized in one hop (longer sem chains stall pass starts).
  - scale/bias: HWDGE stride-0 broadcast to an fp32 transient, DVE-cast to
    bf16 replicas. (A gpsimd SWDGE cast+broadcast here intermittently
    corrupted one psum row of one tile -- rogue descriptor write.)
  - Queue roles: sync = weight XBAR transposes + half the x loads; scalar =
    other x loads + rep loads + PSUM->SBUF copies + output writes; DVE =
    casts + epilogue (psum*scale+bias); PE = matmuls + x transposes.
  - Interleaved (no 0,1) phase with no=0 running two tiles ahead (covers
    wt16(1) prep) and a staggered tail (STAG) so wT2's pool slot frees with
    ~37us of matmul cover; chunk production issued 4 ahead, self-paced by
    the staging pools.
"""

import sys

if "/opt/trn_rl_repo" not in sys.path:
    sys.path.insert(0, "/opt/trn_rl_repo")

import numpy as np

import concourse.bass as bass
import concourse.mybir as mybir
import concourse.tile as tile
from concourse import bacc
from concourse.masks import make_identity

P = 128
N_CORES = 8
B, S, K, N = 4, 4096, 4096, 4096
M_TOTAL = B * S
M_SH = M_TOTAL // N_CORES  # 2048 rows per core
KO = K // P  # 32
KO16 = 24  # k-chunks done in fp16
KO8 = KO - KO16  # 8 k-chunks done in fp8 DoubleRow (4 instructions)
KCUT = KO16 * P  # 3072
MO = M_SH // P  # 16
N_TILE = 512
NO = N // N_TILE  # 8
STAG = 6  # no=0 finishes STAG tiles early to give the wT2 transpose cover

F32 = mybir.dt.float32
F16 = mybir.dt.float16
BF16 = mybir.dt.bfloat16
F8 = mybir.dt.float8e4
DR = mybir.MatmulPerfMode.DoubleRow

_CACHED_NC = None


def _build_nc():
    nc = bacc.Bacc(
        None,
        target_bir_lowering=False,
        num_swdge_queues=1,
        dynamic_dma_scratch_size=2048,
    )

    x = nc.dram_tensor("x", (M_SH, K), F32, kind="ExternalInput")
    w = nc.dram_tensor("weight", (N, K), F16, kind="ExternalInput")
    scale = nc.dram_tensor("scale_weight", (N,), F32, kind="ExternalInput")
    bias = nc.dram_tensor("bias", (N,), F32, kind="ExternalInput")
    # Output is written bf16 and upcast to fp32 on the host (exact widening):
    # halves the output DMA bytes, relieving the HWDGE queues during the
    # x-supply-bound ramp. Costs ~2e-3 rel err in quadrature -> ~1.877e-2.
    out = nc.dram_tensor("out", (M_SH, N), BF16, kind="ExternalOutput")

    with tile.TileContext(nc) as tc:
        with (
            tc.tile_pool(name="xT", bufs=1) as xtp,
            tc.tile_pool(name="x8", bufs=1) as x8p,
            tc.tile_pool(name="wT", bufs=2) as wtp,
            tc.tile_pool(name="w8", bufs=2) as w8p,
            tc.tile_pool(name="x32s", bufs=4) as x32p,
            tc.tile_pool(name="x32y", bufs=3) as x32yp,
            tc.tile_pool(name="x16s", bufs=2) as x16p,
            tc.tile_pool(name="wf8s", bufs=1) as wf8p,
            tc.tile_pool(name="ident", bufs=1) as idp,
            tc.tile_pool(name="sbrep", bufs=2) as sbp,
            tc.tile_pool(name="reptmp", bufs=1) as rtp,
            tc.tile_pool(name="psum", bufs=4, space="PSUM") as pp,
            tc.tile_pool(name="psumT", bufs=2, space="PSUM") as ptp,
            tc.tile_pool(name="osb", bufs=2) as op,
        ):
            # resident transposed operands
            xT = xtp.tile((P, MO, KO16, P), F16)   # xT[p,mo,ko,m] = x16[mo*128+m, ko*128+p]
            x8 = x8p.tile((P, MO, KO8, P), F8)     # fp8 ko 24..31
            ident = idp.tile((P, P), F16)
            make_identity(nc, ident[:])
            wts = {}
            w8s = {}
            sreps = {}

            wf8s = {}

            def make_wf8(no):
                # fp8 part of the weight tile: XBAR-transpose ko 24..31 into an
                # fp16 staging tile (quantized later by quant_w8). All XBAR
                # transposes ride the sync queue: two queues driving the XBAR
                # concurrently corrupts transfers.
                wf8 = wf8p.tile((P, KO8, N_TILE), F16, tag="wf8")
                nc.sync.dma_start_transpose(
                    wf8[:], w[no * N_TILE:(no + 1) * N_TILE, KCUT:K]
                )
                wf8s[no] = wf8

            def make_wt16(no, pieces=2):
                # wT[p,ko,n] = w[no*512+n, ko*128+p]; fp16 part in `pieces`
                # XBAR transposes. (An 8-piece trickle for pass 0 starts the
                # first matmul at ~12us instead of ~23us but measured NET
                # SLOWER overall -- the extra sync kicks delay the chunk
                # banking this schedule relies on. Keep 2.)
                wTn = wtp.tile((P, KO16, N_TILE), F16, tag="wT")
                assert KO16 % pieces == 0
                sko = KO16 // pieces
                for j in range(pieces):
                    nc.sync.dma_start_transpose(
                        wTn[:, sko * j:sko * (j + 1), :],
                        w[no * N_TILE:(no + 1) * N_TILE,
                          sko * P * j:sko * P * (j + 1)],
                    )
                wts[no] = wTn

            def quant_w8(no):
                # DVE fp16 -> fp8e4; emitted separately so its queue position
                # (and the wf8 gate) never blocks epilogues for long.
                w8n = w8p.tile((P, KO8, N_TILE), F8, tag="w8")
                nc.vector.tensor_copy(w8n[:], wf8s[no][:])
                w8s[no] = w8n

            def make_wt(no):
                make_wf8(no)
                make_wt16(no)

            def make_reps(no):
                # HWDGE stride-0 partition broadcast into an fp32 transient,
                # DVE-cast to resident bf16 replicas (saves 4KB vs fp32 reps)
                s_rep = sbp.tile((P, N_TILE), BF16, tag="scale")
                b_rep = sbp.tile((P, N_TILE), BF16, tag="bias")
                for rep, src in ((s_rep, scale), (b_rep, bias)):
                    tmp = rtp.tile((P, N_TILE), F32, tag="rtmp")
                    sl = src[slice(no * N_TILE, (no + 1) * N_TILE)]
                    nc.scalar.dma_start(
                        out=tmp[:],
                        in_=bass.AP(tensor=sl.tensor, offset=sl.offset,
                                    ap=[[0, P], *sl.ap]),
                    )
                    nc.vector.tensor_copy(rep[:], tmp[:])
                sreps[no] = (s_rep, b_rep)

            def loads_chunk(mo, engs=(nc.scalar, nc.sync)):
                # x[mo*128:(mo+1)*128, :] fp32 loaded in 8 pieces (alternating
                # across the given HWDGE queues), DVE-cast to fp16 halves.
                # The ramp is x-supply-bound (every big PE gap in the trace
                # waits on these casts). Each queue's dma_start trigger waits
                # FIFO-blocking on a staging-pool slot (freed by the DVE cast
                # ~5us later), so per-queue throughput = slots-in-flight x
                # 262KB / recycle-latency. Per-queue SPLIT pools (3 scalar +
                # 2 sync slots) decouple the queues and deepen the scalar
                # pipeline. (gpsimd SWDGE as a 3rd queue was tested twice:
                # intermittent NaN corruption both times -- never use it.)
                rows = slice(mo * P, (mo + 1) * P)
                halves = []
                for h in range(2):
                    x16h = x16p.tile((P, K // 2), F16, tag="x16")
                    for q in range(4):
                        j = 4 * h + q
                        eng = engs[j % len(engs)]
                        pool = x32p if eng is nc.scalar else x32yp
                        x32 = pool.tile((P, 512), F32, tag="x32")
                        eng.dma_start(out=x32[:], in_=x[rows, 512 * j:512 * (j + 1)])
                        nc.vector.tensor_copy(
                            x16h[:, 512 * q:512 * (q + 1)], x32[:]
                        )
                    halves.append(x16h)
                return halves

            def produce_chunk(mo, all_scalar=False, halves=None):
                if halves is None:
                    halves = loads_chunk(
                        mo, engs=(nc.scalar,) if all_scalar else (nc.scalar, nc.sync)
                    )
                rows = slice(mo * P, (mo + 1) * P)
                # Transpose on the PE (is_transpose matmuls through PSUM):
                # XBAR cannot be used here -- a DMA-transpose with an SBUF
                # source is the documented "DMA-transpose || SBUF->SBUF DMA"
                # hazard and corrupts transfers under load.
                # half 0 = ko 0..15 (fp16); half 1 = ko 16..25 fp16 + 26..31 fp8
                # 4 groups of 8: regular matmuls against the identity
                # (out = in.T @ I, fp32 PSUM). Identical result/cost to
                # transpose-mode, but counts as PE-busy for the HAM clock
                # gate (transpose-mode doesn't), so ramp bursts re-warm to
                # 2.4GHz sooner. Groups: kos 0-7, 8-15, 16-23 -> xT (fp16),
                # kos 24-31 -> x8 (fp8).
                for g in range(4):
                    pt = ptp.tile((P, 8, P), F32, tag="pt")
                    half = halves[g // 2]
                    base = (g % 2) * 8
                    for j in range(8):
                        nc.tensor.matmul(
                            pt[:, j, :],
                            lhsT=half[:, P * (base + j):P * (base + j + 1)],
                            rhs=ident[:],
                            start=True,
                            stop=True,
                        )
                    if g < 3:
                        nc.scalar.copy(xT[:, mo, 8 * g:8 * (g + 1), :], pt[:])
                    else:
                        nc.vector.tensor_copy(x8[:, mo], pt[:])

            def mm_tile(mo, no):
                ncols = slice(no * N_TILE, (no + 1) * N_TILE)
                wT = wts[no]
                w8n = w8s[no]
                scale_rep, bias_rep = sreps[no]
                ps = pp.tile((P, N_TILE), F32, tag="ps")
                for ko in range(KO16):
                    nc.tensor.matmul(
                        ps[:],
                        lhsT=xT[:, mo, ko, :],
                        rhs=wT[:, ko, :],
                        start=(ko == 0),
                        stop=False,
                    )
                for j in range(KO8 // 2):
                    nc.tensor.matmul(
                        ps[:],
                        lhsT=x8[:, mo, 2 * j:2 * j + 2, :],
                        rhs=w8n[:, 2 * j:2 * j + 2, :],
                        start=False,
                        stop=(j == KO8 // 2 - 1),
                        perf_mode=DR,
                    )
                ot = op.tile((P, N_TILE), BF16, tag="ot")
                nc.vector.tensor_mul(ot[:], ps[:], scale_rep[:])
                nc.vector.tensor_add(ot[:], ot[:], bias_rep[:])
                nc.scalar.dma_start(out[mo * P:(mo + 1) * P, ncols], ot[:])

            def mm_tile_pair(mo, no, width=2):
                # `width` tiles with their fp16 runs back-to-back, then all
                # DR runs. The PE pays ~230-350ns at every normal<->DoubleRow
                # mode boundary (trace: first DR matmul spacing 403-566ns vs
                # the 215ns flow, ~1.3 events/tile = ~38us total); batching
                # divides the transitions by `width`. PSUM: width=2 keeps 2 banks
                # open + 2 draining. (width=4 measured +9us -- the full-bank
                # occupancy stalls the pipeline; keep 2.)
                ncols = slice(no * N_TILE, (no + 1) * N_TILE)
                wT = wts[no]
                w8n = w8s[no]
                scale_rep, bias_rep = sreps[no]
                pss = {}
                for m in range(mo, mo + width):
                    ps = pp.tile((P, N_TILE), F32, tag="ps", name="ps")
                    pss[m] = ps
                    for ko in range(KO16):
                        nc.tensor.matmul(
                            ps[:],
                            lhsT=xT[:, m, ko, :],
                            rhs=wT[:, ko, :],
                            start=(ko == 0),
                            stop=False,
                        )
                for m in range(mo, mo + width):
                    for j in range(KO8 // 2):
                        nc.tensor.matmul(
                            pss[m][:],
                            lhsT=x8[:, m, 2 * j:2 * j + 2, :],
                            rhs=w8n[:, 2 * j:2 * j + 2, :],
                            start=False,
                            stop=(j == KO8 // 2 - 1),
                            perf_mode=DR,
                        )
                for m in range(mo, mo + width):
                    ot = op.tile((P, N_TILE), BF16, tag="ot", name="ot")
                    nc.vector.tensor_mul(ot[:], pss[m][:], scale_rep[:])
                    nc.vector.tensor_add(ot[:], ot[:], bias_rep[:])
                    nc.scalar.dma_start(out[m * P:(m + 1) * P, ncols], ot[:])

            # ---- prologue: wf8(0) first (tiny, the DR matmuls ending tile
            # (0,0) need it), then wt16(0) / wt16(1) pieces on scalar while
            # sync builds chunks 0..3.
            # chunk 0/1 loads race ahead on both queues before the wT
            # transposes occupy sync; the wt16(0) wait (~30us) banks chunks.
            h0 = loads_chunk(0)
            make_wf8(0)
            make_wt16(0)
            quant_w8(0)  # after wt16(0): its wf8b transpose must not delay it
            produce_chunk(0, halves=h0)
            make_reps(0)
            produce_chunk(1, all_scalar=True)
            make_reps(1)
            produce_chunk(2, all_scalar=True)
            make_wt16(1)
            make_wf8(1)

            # ---- interleaved phase over no in {0,1}, with no=0 running two
            # tiles ahead so the first no=1 tile lands after wt16(1)'s
            # transposes. Chunk consumption is ~12.4us per pair; production is
            # issued 2-4 chunks ahead (~7.5us/chunk of queue time on each of
            # sync/scalar).
            mm_tile(0, 0)
            quant_w8(1)
            produce_chunk(3)
            mm_tile(1, 0)
            produce_chunk(4)
            for mo in range(MO - STAG):
                mm_tile(mo + 2, 0)
                mm_tile(mo, 1)
                if mo + 5 < MO:
                    produce_chunk(mo + 5)
                if mo >= 9:
                    produce_chunk(mo + 6)  # chunk 15 early
            # staggered tail: finish no=0 (tiles 12..15), so wT2's pool slot
            # frees with 6 no=1 tiles (~37us) of matmul cover.
            for mo in range(MO - STAG + 2, MO):
                mm_tile(mo, 0)
            make_wt(2)
            make_reps(2)
            for i, mo in enumerate(range(MO - STAG, MO)):
                mm_tile(mo, 1)
                if i == 2:
                    quant_w8(2)

            # ---- remaining passes; wT(no+1) + reps(no+1) issued at the start
            # of pass no so their transposes run with a full pass of cover;
            # the w8 quant a few tiles in so the wf8 gate never backs up the
            # DVE epilogue stream.
            for no in range(2, NO):
                if no + 1 < NO:
                    make_wt(no + 1)
                    make_reps(no + 1)
                for mo in range(0, MO, 2):
                    mm_tile_pair(mo, no)
                    if mo == 2 and no + 1 < NO:
                        quant_w8(no + 1)

    nc.finalize()
    return nc


def _get_nc():
    global _CACHED_NC
    if _CACHED_NC is None:
        _CACHED_NC = _build_nc()
    return _CACHED_NC


def _run(inputs, trace=False, **spmd_kwargs):
    from concourse.bass_utils import run_bass_kernel_spmd

    x = np.asarray(inputs["x"], dtype=np.float32).reshape(M_TOTAL, K)
    w = np.ascontiguousarray(np.asarray(inputs["weight"], dtype=np.float16))
    scale = np.ascontiguousarray(np.asarray(inputs["scale_weight"], dtype=np.float32))
    bias = np.ascontiguousarray(np.asarray(inputs["bias"], dtype=np.float32))

    in_maps = []
    for c in range(N_CORES):
        in_maps.append(
            {
                "x": np.ascontiguousarray(x[c * M_SH:(c + 1) * M_SH]),
                "weight": w,
                "scale_weight": scale,
                "bias": bias,
            }
        )

    nc = _get_nc()
    res = run_bass_kernel_spmd(
        nc, in_maps, core_ids=list(range(N_CORES)), trace=trace, **spmd_kwargs
    )
    out = np.concatenate([res.results[c]["out"] for c in range(N_CORES)], axis=0).astype(np.float32)
    return out.reshape(B, S, N), res


def kernel(x, weight, scale_weight, bias):
    out, _ = _run({"x": x, "weight": weight, "scale_weight": scale_weight, "bias": bias})
    return out



# revision 48
# speedup vs baseline: 1.1777x; 1.0051x over previous
"""FP8ScaledLayer kernel for Trainium2 (8 NeuronCores, SPMD data-parallel).

Computes out = x @ (weight * scale[:, None]).T + bias with
  x: [4, 4096, 4096] fp32, weight: [4096, 4096] fp16,
  scale_weight: [4096] fp32, bias: [4096] fp32  ->  out [4, 4096, 4096] fp32.

Sharding: data-parallel over tokens (B*S = 16384 -> 2048 rows/core).
Weight is replicated; x is sharded, keeping every core compute-bound.

Design (evolved over several trace-driven iterations):
  - x path stays entirely on-chip: straight fp32 piece-loads into SBUF
    staging (split per-queue pools), DVE cast fp32->fp16, then PE transposes
    into the K-major resident xT/x8. The transposes are REGULAR matmuls
    against the identity (out = in.T @ I, fp32 PSUM, 4 groups of 8 per
    chunk) rather than transpose-mode: same result and cost, but they count
    as PE-busy for the HAM clock gate, so ramp bursts re-warm to 2.4GHz
    sooner (measured -7us). The XBAR must NOT be used for SBUF-source
    transposes: "DMA-transpose || SBUF->SBUF DMA" is a documented HW hazard
    and corrupts transfers under load. gpsimd SWDGE as a 3rd DMA queue was
    tested twice and produced intermittent NaN corruption both times --
    never use it here.
  - Mixed-precision K split: 24 of 32 k-chunks run fp16 matmuls, the last 8
    run as 4 fp8e4 DoubleRow matmuls (2x rate; each fp16->DR pair swap saves
    ~185ns/tile, ~24us total). Exact deterministic rel_err measured on HW
    with the real inputs: 1.890e-2 incl the bf16 output rounding (gate
    2e-2; offline e4m3 emulation predicts 1.866e-2 for the fp8 part alone
    and tracked the 6-chunk variant to ~1%; 9 chunks would land at
    ~1.98e-2 -- too close to the gate). The SBUF freed by the
    smaller xT is what lets the staging fit.
  - Weight tiles stream per 512-col pass: XBAR transposes (DRAM source only)
    on the sync queue, prep issued a full pass ahead; the fp8 ko's staged
    fp16 then DVE-quantized in one hop (longer sem chains stall pass starts).
  - scale/bias: HWDGE stride-0 broadcast to an fp32 transient, DVE-cast to
    bf16 replicas. (A gpsimd SWDGE cast+broadcast here intermittently
    corrupted one psum row of one tile -- rogue descriptor write.)
  - Queue roles: sync = weight XBAR transposes + half the x loads; scalar =
    other x loads + rep loads + PSUM->SBUF copies + output writes; DVE =
    casts + epilogue (psum*scale+bias); PE = matmuls + x transposes.
  - Interleaved (no 0,1) phase with no=0 running two tiles ahead (covers
    wt16(1) prep) and a staggered tail (STAG) so wT2's pool slot frees with
    ~37us of matmul cover; chunk production issued 4 ahead, self-paced by
    the staging pools.
"""

import sys

if "/opt/trn_rl_repo" not in sys.path:
    sys.path.insert(0, "/opt/trn_rl_repo")

import numpy as np

import concourse.bass as bass
import concourse.mybir as mybir
import concourse.tile as tile
from concourse import bacc
from concourse.masks import make_identity

P = 128
N_CORES = 8
B, S, K, N = 4, 4096, 4096, 4096
M_TOTAL = B * S
M_SH = M_TOTAL // N_CORES  # 2048 rows per core
KO = K // P  # 32
KO16 = 24  # k-chunks done in fp16
KO8 = KO - KO16  # 8 k-chunks done in fp8 DoubleRow (4 instructions)
KCUT = KO16 * P  # 3072
MO = M_SH // P  # 16
N_TILE = 512
NO = N // N_TILE  # 8
STAG = 6  # no=0 finishes STAG tiles early to give the wT2 transpose cover

F32 = mybir.dt.float32
F16 = mybir.dt.float16
BF16 = mybir.dt.bfloat16
F8 = mybir.dt.float8e4
DR = mybir.MatmulPerfMode.DoubleRow

_CACHED_NC = None


def _build_nc():
    nc = bacc.Bacc(
        None,
        target_bir_lowering=False,
        num_swdge_queues=1,
        dynamic_dma_scratch_size=2048,
    )

    x = nc.dram_tensor("x", (M_SH, K), F32, kind="ExternalInput")
    w = nc.dram_tensor("weight", (N, K), F16, kind="ExternalInput")
    scale = nc.dram_tensor("scale_weight", (N,), F32, kind="ExternalInput")
    bias = nc.dram_tensor("bias", (N,), F32, kind="ExternalInput")
    # Output is written bf16 and upcast to fp32 on the host (exact widening):
    # halves the output DMA bytes, relieving the HWDGE queues during the
    # x-supply-bound ramp. Costs ~2e-3 rel err in quadrature -> ~1.877e-2.
    out = nc.dram_tensor("out", (M_SH, N), BF16, kind="ExternalOutput")

    with tile.TileContext(nc) as tc:
        with (
            tc.tile_pool(name="xT", bufs=1) as xtp,
            tc.tile_pool(name="x8", bufs=1) as x8p,
            tc.tile_pool(name="wT", bufs=2) as wtp,
            tc.tile_pool(name="w8", bufs=2) as w8p,
            tc.tile_pool(name="x32s", bufs=4) as x32p,
            tc.tile_pool(name="x32y", bufs=3) as x32yp,
            tc.tile_pool(name="x16s", bufs=2) as x16p,
            tc.tile_pool(name="wf8s", bufs=1) as wf8p,
            tc.tile_pool(name="ident", bufs=1) as idp,
            tc.tile_pool(name="sbrep", bufs=2) as sbp,
            tc.tile_pool(name="reptmp", bufs=1) as rtp,
            tc.tile_pool(name="psum", bufs=4, space="PSUM") as pp,
            tc.tile_pool(name="psumT", bufs=2, space="PSUM") as ptp,
            tc.tile_pool(name="osb", bufs=2) as op,
        ):
            # resident transposed operands
            xT = xtp.tile((P, MO, KO16, P), F16)   # xT[p,mo,ko,m] = x16[mo*128+m, ko*128+p]
            x8 = x8p.tile((P, MO, KO8, P), F8)     # fp8 ko 24..31
            ident = idp.tile((P, P), F16)
            make_identity(nc, ident[:])
            wts = {}
            w8s = {}
            sreps = {}

            wf8s = {}

            def make_wf8(no):
                # fp8 part of the weight tile: XBAR-transpose ko 24..31 into an
                # fp16 staging tile (quantized later by quant_w8). All XBAR
                # transposes ride the sync queue: two queues driving the XBAR
                # concurrently corrupts transfers.
                wf8 = wf8p.tile((P, KO8, N_TILE), F16, tag="wf8")
                nc.sync.dma_start_transpose(
                    wf8[:], w[no * N_TILE:(no + 1) * N_TILE, KCUT:K]
                )
                wf8s[no] = wf8

            def make_wt16(no, pieces=2):
                # wT[p,ko,n] = w[no*512+n, ko*128+p]; fp16 part in `pieces`
                # XBAR transposes. (An 8-piece trickle for pass 0 starts the
                # first matmul at ~12us instead of ~23us but measured NET
                # SLOWER overall -- the extra sync kicks delay the chunk
                # banking this schedule relies on. Keep 2.)
                wTn = wtp.tile((P, KO16, N_TILE), F16, tag="wT")
                assert KO16 % pieces == 0
                sko = KO16 // pieces
                for j in range(pieces):
                    nc.sync.dma_start_transpose(
                        wTn[:, sko * j:sko * (j + 1), :],
                        w[no * N_TILE:(no + 1) * N_TILE,
                          sko * P * j:sko * P * (j + 1)],
                    )
                wts[no] = wTn

            def quant_w8(no):
                # DVE fp16 -> fp8e4; emitted separately so its queue position
                # (and the wf8 gate) never blocks epilogues for long.
                w8n = w8p.tile((P, KO8, N_TILE), F8, tag="w8")
                nc.vector.tensor_copy(w8n[:], wf8s[no][:])
                w8s[no] = w8n

            def make_wt(no):
                make_wf8(no)
                make_wt16(no)

            def make_reps(no):
                # HWDGE stride-0 partition broadcast into an fp32 transient,
                # DVE-cast to resident bf16 replicas (saves 4KB vs fp32 reps)
                s_rep = sbp.tile((P, N_TILE), BF16, tag="scale")
                b_rep = sbp.tile((P, N_TILE), BF16, tag="bias")
                for rep, src in ((s_rep, scale), (b_rep, bias)):
                    tmp = rtp.tile((P, N_TILE), F32, tag="rtmp")
                    sl = src[slice(no * N_TILE, (no + 1) * N_TILE)]
                    nc.scalar.dma_start(
                        out=tmp[:],
                        in_=bass.AP(tensor=sl.tensor, offset=sl.offset,
                                    ap=[[0, P], *sl.ap]),
                    )
                    nc.vector.tensor_copy(rep[:], tmp[:])
                sreps[no] = (s_rep, b_rep)

            def loads_chunk(mo, engs=(nc.scalar, nc.sync)):
                # x[mo*128:(mo+1)*128, :] fp32 loaded in 8 pieces (alternating
                # across the given HWDGE queues), DVE-cast to fp16 halves.
                # The ramp is x-supply-bound (every big PE gap in the trace
                # waits on these casts). Each queue's dma_start trigger waits
                # FIFO-blocking on a staging-pool slot (freed by the DVE cast
                # ~5us later), so per-queue throughput = slots-in-flight x
                # 262KB / recycle-latency. Per-queue SPLIT pools (3 scalar +
                # 2 sync slots) decouple the queues and deepen the scalar
                # pipeline. (gpsimd SWDGE as a 3rd queue was tested twice:
                # intermittent NaN corruption both times -- never use it.)
                rows = slice(mo * P, (mo + 1) * P)
                halves = []
                for h in range(2):
                    x16h = x16p.tile((P, K // 2), F16, tag="x16")
                    for q in range(4):
                        j = 4 * h + q
                        eng = engs[j % len(engs)]
                        pool = x32p if eng is nc.scalar else x32yp
                        x32 = pool.tile((P, 512), F32, tag="x32")
                        eng.dma_start(out=x32[:], in_=x[rows, 512 * j:512 * (j + 1)])
                        nc.vector.tensor_copy(
                            x16h[:, 512 * q:512 * (q + 1)], x32[:]
                        )
                    halves.append(x16h)
                return halves

            def produce_chunk(mo, all_scalar=False, halves=None):
                if halves is None:
                    halves = loads_chunk(
                        mo, engs=(nc.scalar,) if all_scalar else (nc.scalar, nc.sync)
                    )
                rows = slice(mo * P, (mo + 1) * P)
                # Transpose on the PE (is_transpose matmuls through PSUM):
                # XBAR cannot be used here -- a DMA-transpose with an SBUF
                # source is the documented "DMA-transpose || SBUF->SBUF DMA"
                # hazard and corrupts transfers under load.
                # half 0 = ko 0..15 (fp16); half 1 = ko 16..25 fp16 + 26..31 fp8
                # 4 groups of 8: regular matmuls against the identity
                # (out = in.T @ I, fp32 PSUM). Identical result/cost to
                # transpose-mode, but counts as PE-busy for the HAM clock
                # gate (transpose-mode doesn't), so ramp bursts re-warm to
                # 2.4GHz sooner. Groups: kos 0-7, 8-15, 16-23 -> xT (fp16),
                # kos 24-31 -> x8 (fp8).
                for g in range(4):
                    pt = ptp.tile((P, 8, P), F32, tag="pt")
                    half = halves[g // 2]
                    base = (g % 2) * 8
                    for j in range(8):
                        nc.tensor.matmul(
                            pt[:, j, :],
                            lhsT=half[:, P * (base + j):P * (base + j + 1)],
                            rhs=ident[:],
                            start=True,
                            stop=True,
                        )
                    if g < 3:
                        nc.scalar.copy(xT[:, mo, 8 * g:8 * (g + 1), :], pt[:])
                    else:
                        nc.vector.tensor_copy(x8[:, mo], pt[:])

            def mm_tile(mo, no):
                ncols = slice(no * N_TILE, (no + 1) * N_TILE)
                wT = wts[no]
                w8n = w8s[no]
                scale_rep, bias_rep = sreps[no]
                ps = pp.tile((P, N_TILE), F32, tag="ps")
                for ko in range(KO16):
                    nc.tensor.matmul(
                        ps[:],
                        lhsT=xT[:, mo, ko, :],
                        rhs=wT[:, ko, :],
                        start=(ko == 0),
                        stop=False,
                    )
                for j in range(KO8 // 2):
                    nc.tensor.matmul(
                        ps[:],
                        lhsT=x8[:, mo, 2 * j:2 * j + 2, :],
                        rhs=w8n[:, 2 * j:2 * j + 2, :],
                        start=False,
                        stop=(j == KO8 // 2 - 1),
                        perf_mode=DR,
                    )
                ot = op.tile((P, N_TILE), BF16, tag="ot")
                nc.vector.tensor_mul(ot[:], ps[:], scale_rep[:])
                nc.vector.tensor_add(ot[:], ot[:], bias_rep[:])
                nc.scalar.dma_start(out[mo * P:(mo + 1) * P, ncols], ot[:])

            def mm_tile_pair(mo, no, width=2):
                # `width` tiles with their fp16 runs back-to-back, then all
                # DR runs. The PE pays ~230-350ns at every normal<->DoubleRow
                # mode boundary (trace: first DR matmul spacing 403-566ns vs
                # the 215ns flow, ~1.3 events/tile = ~38us total); batching
                # divides the transitions by `width`. PSUM: width=2 keeps 2 banks
                # open + 2 draining. (width=4 measured +9us -- the full-bank
                # occupancy stalls the pipeline; keep 2.)
                ncols = slice(no * N_TILE, (no + 1) * N_TILE)
                wT = wts[no]
                w8n = w8s[no]
                scale_rep, bias_rep = sreps[no]
                pss = {}
                for m in range(mo, mo + width):
                    ps = pp.tile((P, N_TILE), F32, tag="ps", name="ps")
                    pss[m] = ps
                    for ko in range(KO16):
                        nc.tensor.matmul(
                            ps[:],
                            lhsT=xT[:, m, ko, :],
                            rhs=wT[:, ko, :],
                            start=(ko == 0),
                            stop=False,
                        )
                for m in range(mo, mo + width):
                    for j in range(KO8 // 2):
                        nc.tensor.matmul(
                            pss[m][:],
                            lhsT=x8[:, m, 2 * j:2 * j + 2, :],
                            rhs=w8n[:, 2 * j:2 * j + 2, :],
                            start=False,
                            stop=(j == KO8 // 2 - 1),
                            perf_mode=DR,
                        )
                for m in range(mo, mo + width):
                    ot = op.tile((P, N_TILE), BF16, tag="ot", name="ot")
                    nc.vector.tensor_mul(ot[:], pss[m][:], scale_rep[:])
                    nc.vector.tensor_add(ot[:], ot[:], bias_rep[:])
                    nc.scalar.dma_start(out[m * P:(m + 1) * P, ncols], ot[:])

            # ---- prologue: wf8(0) first (tiny, the DR matmuls ending tile
            # (0,0) need it), then wt16(0) / wt16(1) pieces on scalar while
            # sync builds chunks 0..3.
            # chunk 0/1 loads race ahead on both queues before the wT
            # transposes occupy sync; the wt16(0) wait (~30us) banks chunks.
            h0 = loads_chunk(0)
            make_wf8(0)
            make_wt16(0)
            quant_w8(0)  # after wt16(0): its wf8b transpose must not delay it
            produce_chunk(0, halves=h0)
            make_reps(0)
            produce_chunk(1, all_scalar=True)
            make_reps(1)
            produce_chunk(2, all_scalar=True)
            make_wt16(1)
            make_wf8(1)

            # ---- interleaved phase over no in {0,1}, with no=0 running two
            # tiles ahead so the first no=1 tile lands after wt16(1)'s
            # transposes. Chunk consumption is ~12.4us per pair; production is
            # issued 2-4 chunks ahead (~7.5us/chunk of queue time on each of
            # sync/scalar).
            mm_tile(0, 0)
            quant_w8(1)
            produce_chunk(3)
            mm_tile(1, 0)
            produce_chunk(4)
            for mo in range(MO - STAG):
                mm_tile(mo + 2, 0)
                mm_tile(mo, 1)
                if mo + 5 < MO:
                    produce_chunk(mo + 5)
                if mo >= 9:
                    produce_chunk(mo + 6)  # chunk 15 early
            # staggered tail: finish no=0 (tiles 12..15), so wT2's pool slot
            # frees with 6 no=1 tiles (~37us) of matmul cover.
            for mo in range(MO - STAG + 2, MO):
                mm_tile(mo, 0)
            make_wt(2)
            make_reps(2)
            for i, mo in enumerate(range(MO - STAG, MO)):
                mm_tile(mo, 1)
                if i == 2:
                    quant_w8(2)

            # ---- remaining passes; wT(no+1) + reps(no+1) issued at the start
            # of pass no so their transposes run with a full pass of cover;
            # the w8 quant a few tiles in so the wf8 gate never backs up the
            # DVE epilogue stream.
            for no in range(2, NO):
                if no + 1 < NO:
                    make_wt(no + 1)
                    make_reps(no + 1)
                for i, (mo, wd) in enumerate(((0, 3), (3, 3), (6, 3), (9, 3), (12, 2), (14, 2))):
                    mm_tile_pair(mo, no, width=wd)
                    if i == 1 and no + 1 < NO:
                        quant_w8(no + 1)

    nc.finalize()
    return nc


def _get_nc():
    global _CACHED_NC
    if _CACHED_NC is None:
        _CACHED_NC = _build_nc()
    return _CACHED_NC


def _run(inputs, trace=False, **spmd_kwargs):
    from concourse.bass_utils import run_bass_kernel_spmd

    x = np.asarray(inputs["x"], dtype=np.float32).reshape(M_TOTAL, K)
    w = np.ascontiguousarray(np.asarray(inputs["weight"], dtype=np.float16))
    scale = np.ascontiguousarray(np.asarray(inputs["scale_weight"], dtype=np.float32))
    bias = np.ascontiguousarray(np.asarray(inputs["bias"], dtype=np.float32))

    in_maps = []
    for c in range(N_CORES):
        in_maps.append(
            {
                "x": np.ascontiguousarray(x[c * M_SH:(c + 1) * M_SH]),
                "weight": w,
                "scale_weight": scale,
                "bias": bias,
            }
        )

    nc = _get_nc()
    res = run_bass_kernel_spmd(
        nc, in_maps, core_ids=list(range(N_CORES)), trace=trace, **spmd_kwargs
    )
    out = np.concatenate([res.results[c]["out"] for c in range(N_CORES)], axis=0).astype(np.float32)
    return out.reshape(B, S, N), res


def kernel(x, weight, scale_weight, bias):
    out, _ = _run({"x": x, "weight": weight, "scale_weight": scale_weight, "bias": bias})
    return out

